# revision 18
# baseline (speedup 1.0000x reference)
"""Multi-head causal attention (B=2, S=2048, D=1024, H=16) on 8 TRN2 NeuronCores.

Sharding: Megatron-style head parallelism. Core c owns heads {2c, 2c+1}:
  - W_q/W_k/W_v column slices [:, 128c:128(c+1)]  (2 heads x 64 dims)
  - attention for those heads over the full sequence (causal)
  - normalized context slices are AllGathered across cores (bf16)
  - each core computes the output-projection column slice
    out[:, 128c:128(c+1)] = ctx_full @ W_o[:, 128c:128(c+1)]
  - host concatenates the 8 column slices (pure gather, no arithmetic)

Compute dtype: bf16 operands, fp32 PSUM accumulation. Scores are computed
transposed (S^T[k,q] = K Q^T) so the P^T tiles feed the A@V matmul directly;
softmax denominators come from an extra all-ones column appended to V.
"""

import os
import numpy as np

import concourse.bass as bass
import concourse.mybir as mybir
from concourse import bacc, tile
from concourse.bass_utils import run_bass_kernel_spmd

N_CORES = 8
B, S, D = 2, 2048, 1024
H, DH = 16, 64
BS = B * S  # 4096
HPC = H // N_CORES  # heads per core = 2
DHC = HPC * DH  # 128 context dims per core
SCALE = 1.0 / 32.0  # 1/sqrt(D)
FP32 = mybir.dt.float32
BF16 = mybir.dt.bfloat16
Exp = mybir.ActivationFunctionType.Exp

NQ = 4  # q macro tiles of 512 per batch element
QM = S // NQ  # 512
NKT = S // 128  # 16 k-tiles of 128 per batch element

_nc_cache = {}
DEBUG_TAPS = False


def _build():
    nc = bacc.Bacc(
        "TRN2", target_bir_lowering=False, debug=False, num_devices=N_CORES
    )

    x_d = nc.dram_tensor("x", [BS, D], FP32, kind="ExternalInput").ap()
    wq_d = nc.dram_tensor("wq", [D, DHC], FP32, kind="ExternalInput").ap()
    wk_d = nc.dram_tensor("wk", [D, DHC], FP32, kind="ExternalInput").ap()
    wv_d = nc.dram_tensor("wv", [D, DHC], FP32, kind="ExternalInput").ap()
    wo_d = nc.dram_tensor("wo", [D, DHC], FP32, kind="ExternalInput").ap()
    tri_d = nc.dram_tensor("tri", [128, 128], FP32, kind="ExternalInput").ap()
    out_d = nc.dram_tensor("out", [BS, DHC], FP32, kind="ExternalOutput").ap()
    dbg = {}
    if DEBUG_TAPS:
        dbg["xt"] = nc.dram_tensor("dbg_xt", [128, 8, BS], BF16, kind="ExternalOutput").ap()
        dbg["qt0"] = nc.dram_tensor("dbg_qt0", [128, S], BF16, kind="ExternalOutput").ap()
        dbg["kt0"] = nc.dram_tensor("dbg_kt0", [128, S], BF16, kind="ExternalOutput").ap()
        dbg["v0"] = nc.dram_tensor("dbg_v0", [128, NKT * 130], BF16, kind="ExternalOutput").ap()
        dbg["ctxin"] = nc.dram_tensor("dbg_ctxin", [DHC, BS], BF16, kind="ExternalOutput").ap()
        dbg["ctxall"] = nc.dram_tensor("dbg_ctxall", [N_CORES * DHC, BS], BF16, kind="ExternalOutput").ap()
        dbg["sps"] = nc.dram_tensor("dbg_sps", [128, 2 * QM], FP32, kind="ExternalOutput").ap()
        dbg["pt"] = nc.dram_tensor("dbg_pt", [128, 2 * QM], BF16, kind="ExternalOutput").ap()
        dbg["ctxps"] = nc.dram_tensor("dbg_ctxps", [65, QM], FP32, kind="ExternalOutput").ap()
        dbg["bcast"] = nc.dram_tensor("dbg_bcast", [64, QM], FP32, kind="ExternalOutput").ap()

    with tile.TileContext(nc) as tc:
        with (
            tc.tile_pool(name="dram", bufs=1, space="DRAM") as dram,
            tc.tile_pool(name="pers", bufs=1) as pers,
            tc.tile_pool(name="ldx", bufs=3) as ldx,
            tc.tile_pool(name="ptp", bufs=4) as ptp,
            tc.tile_pool(name="nw", bufs=3) as nw,
            tc.tile_pool(name="ps_s", bufs=2, space="PSUM") as ps_s,
            tc.tile_pool(name="ps_c", bufs=2, space="PSUM") as ps_c,
            tc.tile_pool(name="ps_m", bufs=2, space="PSUM") as ps_m,
        ):
            # ---- persistent SBUF ----
            qt_sb = [pers.tile([128, S], BF16, name=f"qt{b}") for b in range(B)]
            kt_sb = [pers.tile([128, S], BF16, name=f"kt{b}") for b in range(B)]
            # V tiles: per k-tile layout [h0 64 | ones | h1 64 | ones] (130 cols)
            v_sb = [pers.tile([128, NKT * 130], BF16, name=f"v{b}") for b in range(B)]
            wq_sb = pers.tile([128, 8, DHC], BF16, name="wq_sb")
            wk_sb = pers.tile([128, 8, DHC], BF16, name="wk_sb")
            wv_sb = pers.tile([128, 8, DHC], BF16, name="wv_sb")
            wo_sb = pers.tile([128, 8, DHC], BF16, name="wo_sb")
            tri_sb = pers.tile([128, 128], BF16, name="tri_sb")

            # ---- weights: load fp32, cast to bf16 ----
            wtmp = pers.tile([128, 8, DHC], FP32, name="wtmp")
            for w_d, w_sb in ((wq_d, wq_sb), (wk_d, wk_sb), (wv_d, wv_sb), (wo_d, wo_sb)):
                for dt in range(8):
                    nc.sync.dma_start(wtmp[:, dt, :], w_d[dt * 128 : (dt + 1) * 128, :])
                nc.vector.tensor_copy(w_sb[:], wtmp[:])
            tri_f = nw.tile([128, 128], FP32, name="tri_f")
            nc.sync.dma_start(tri_f[:], tri_d[:])
            nc.vector.tensor_copy(tri_sb[:], tri_f[:])

            # ---- x: load fp32, cast bf16, write back, transpose-read ----
            xbf_dram = dram.tile([BS, D], BF16, name="xbf_dram")
            with tc.tile_pool(name="xtp", bufs=1) as xtp:
                xt_sb = xtp.tile([128, 8, BS], BF16, name="xt_sb")
                for st in range(BS // 128):
                    x_f = ldx.tile([128, D], FP32, name="x_f")
                    nc.sync.dma_start(x_f[:], x_d[st * 128 : (st + 1) * 128, :])
                    x_b = ldx.tile([128, D], BF16, name="x_b")
                    eng = nc.scalar if st % 2 == 0 else nc.vector
                    if eng is nc.scalar:
                        eng.copy(x_b[:], x_f[:])
                    else:
                        eng.tensor_copy(x_b[:], x_f[:])
                    nc.sync.dma_start(xbf_dram[st * 128 : (st + 1) * 128, :], x_b[:])
                for sc in range(BS // 512):
                    for dt in range(8):
                        nc.sync.dma_start_transpose(
                            xt_sb[:, dt, sc * 512 : (sc + 1) * 512],
                            xbf_dram[sc * 512 : (sc + 1) * 512, dt * 128 : (dt + 1) * 128],
                        )

                # ---- QKV projections ----
                for b in range(B):
                    nc.gpsimd.memset(v_sb[b][:], 1.0)
                    for j in range(NQ):
                        cols = slice(b * S + j * QM, b * S + (j + 1) * QM)
                        for w_sb, t_sb in ((wq_sb, qt_sb[b]), (wk_sb, kt_sb[b])):
                            ps = ps_m.tile([128, QM], FP32, name="ps_qk", tag="m")
                            for dt in range(8):
                                nc.tensor.matmul(
                                    ps[:],
                                    w_sb[:, dt, :],
                                    xt_sb[:, dt, cols],
                                    start=(dt == 0),
                                    stop=(dt == 7),
                                )
                            nc.vector.tensor_copy(t_sb[:, j * QM : (j + 1) * QM], ps[:])
                        # V natural: per 128-row k-tile
                        for st2 in range(4):
                            kt_i = j * 4 + st2
                            scol = slice(
                                b * S + kt_i * 128, b * S + (kt_i + 1) * 128
                            )
                            ps_v = ps_m.tile([128, QM], FP32, name="ps_v", tag="m")
                            for dt in range(8):
                                nc.tensor.matmul(
                                    ps_v[:, 0:DHC],
                                    xt_sb[:, dt, scol],
                                    wv_sb[:, dt, :],
                                    start=(dt == 0),
                                    stop=(dt == 7),
                                )
                            dst = v_sb[b][:, kt_i * 130 : kt_i * 130 + 130].rearrange(
                                "p (g c) -> p g c", g=2
                            )[:, :, 0:64]
                            src = ps_v[:, 0:DHC].rearrange("p (g c) -> p g c", g=2)
                            nc.vector.tensor_copy(dst, src)
                if DEBUG_TAPS:
                    nc.sync.dma_start(dbg["xt"][:], xt_sb[:])
                    nc.sync.dma_start(dbg["qt0"][:], qt_sb[0][:])
                    nc.sync.dma_start(dbg["kt0"][:], kt_sb[0][:])
                    nc.sync.dma_start(dbg["v0"][:], v_sb[0][:])

            # ---- attention (transposed scores, flash-style) ----
            ctx_in = dram.tile([DHC, BS], BF16, name="ctx_in")
            for b in range(B):
                for m in range(NQ):
                    qcols = slice(m * QM, (m + 1) * QM)
                    ctx_ps = [
                        ps_c.tile([65, QM], FP32, name=f"ctx_ps{h}", tag="c") for h in range(HPC)
                    ]
                    n_kt = 4 * m + 4
                    for kt in range(n_kt):
                        s_ps = ps_s.tile([128, 2 * QM], FP32, name="s_ps", tag="s")
                        for h in range(HPC):
                            nc.tensor.matmul(
                                s_ps[:, h * QM : (h + 1) * QM],
                                kt_sb[b][h * 64 : (h + 1) * 64, kt * 128 : (kt + 1) * 128],
                                qt_sb[b][h * 64 : (h + 1) * 64, qcols],
                                start=True,
                                stop=True,
                                tile_position=(h * 64, 0),
                            )
                        pt = ptp.tile([128, 2 * QM], BF16, name="pt")
                        j = kt - 4 * m  # diagonal block index if >= 0
                        if j < 0:
                            # fully-causal tile: one exp over both heads
                            nc.scalar.activation(pt[:], s_ps[:], Exp, scale=SCALE)
                        else:
                            qs = 128 * j
                            for h in range(HPC):
                                nc.scalar.activation(
                                    pt[:, h * QM + qs : (h + 1) * QM],
                                    s_ps[:, h * QM + qs : (h + 1) * QM],
                                    Exp,
                                    scale=SCALE,
                                )
                                # triangular mask on the diagonal 128-col block
                                nc.vector.tensor_mul(
                                    pt[:, h * QM + qs : h * QM + qs + 128],
                                    pt[:, h * QM + qs : h * QM + qs + 128],
                                    tri_sb[:],
                                )
                        if DEBUG_TAPS and b == 0 and m == 0 and kt == 0:
                            sps_cp = nw.tile([128, 2 * QM], FP32, name="sps_cp")
                            nc.vector.tensor_copy(sps_cp[:], s_ps[:])
                            nc.sync.dma_start(dbg["sps"][:], sps_cp[:])
                            nc.sync.dma_start(dbg["pt"][:], pt[:])
                        qs = max(0, 128 * (kt - 4 * m))
                        for h in range(HPC):
                            nc.tensor.matmul(
                                ctx_ps[h][:, qs:QM],
                                v_sb[b][:, kt * 130 + h * 65 : kt * 130 + (h + 1) * 65],
                                pt[:, h * QM + qs : (h + 1) * QM],
                                start=(kt == 0),
                                stop=(kt == n_kt - 1),
                            )
                    # normalize: ctx[0:64] * (1 / rowsum) ; rowsum in row 64
                    if DEBUG_TAPS and b == 0 and m == 0:
                        cps_cp = nw.tile([65, QM], FP32, name="cps_cp")
                        nc.vector.tensor_copy(cps_cp[:], ctx_ps[0][:])
                        nc.sync.dma_start(dbg["ctxps"][:], cps_cp[:])
                    for h in range(HPC):
                        recip = nw.tile([1, QM], FP32, name="recip")
                        nc.vector.reciprocal(recip[:], ctx_ps[h][64:65, :])
                        bcast = nw.tile([64, QM], FP32, name="bcast")
                        nc.gpsimd.partition_broadcast(bcast[:], recip[:])
                        if DEBUG_TAPS and b == 0 and m == 0 and h == 0:
                            nc.sync.dma_start(dbg["bcast"][:], bcast[:])
                        ctxn = nw.tile([64, QM], BF16, name="ctxn")
                        nc.vector.tensor_mul(ctxn[:], ctx_ps[h][0:64, :], bcast[:])
                        nc.sync.dma_start(
                            ctx_in[
                                h * 64 : (h + 1) * 64,
                                b * S + m * QM : b * S + (m + 1) * QM,
                            ],
                            ctxn[:],
                        )

            # ---- AllGather context slices ----
            ctx_all = dram.tile(
                [N_CORES * DHC, BS], BF16, name="ctx_all", addr_space="Shared"
            )
            nc.gpsimd.collective_compute(
                "AllGather",
                mybir.AluOpType.bypass,
                replica_groups=[list(range(N_CORES))],
                ins=[ctx_in[:]],
                outs=[ctx_all[:]],
            )
            if DEBUG_TAPS:
                nc.sync.dma_start(dbg["ctxin"][:], ctx_in[:])
                nc.sync.dma_start(dbg["ctxall"][:], ctx_all[:])

            # ---- output projection: out[:, c-slice] = ctx_full @ wo_slice ----
            with tc.tile_pool(name="cfp", bufs=1) as cfp:
                cf_sb = cfp.tile([128, 8, BS], BF16, name="cf_sb")
                for dt in range(8):
                    nc.sync.dma_start(
                        cf_sb[:, dt, :], ctx_all[dt * 128 : (dt + 1) * 128, :]
                    )
                for qt in range(BS // 128):
                    ps_o = ps_m.tile([128, QM], FP32, name="ps_o", tag="m")
                    for dt in range(8):
                        nc.tensor.matmul(
                            ps_o[:, 0:DHC],
                            cf_sb[:, dt, qt * 128 : (qt + 1) * 128],
                            wo_sb[:, dt, :],
                            start=(dt == 0),
                            stop=(dt == 7),
                        )
                    o_sb = nw.tile([128, DHC], FP32, name="o_sb")
                    nc.scalar.copy(o_sb[:], ps_o[:, 0:DHC])
                    nc.sync.dma_start(out_d[qt * 128 : (qt + 1) * 128, :], o_sb[:])

    nc.compile()
    return nc


def _build_nc():
    if "nc" not in _nc_cache:
        _nc_cache["nc"] = _build()
    return _nc_cache["nc"]


def kernel(x, W_q, W_k, W_v, W_o):
    x = np.ascontiguousarray(np.asarray(x, dtype=np.float32)).reshape(BS, D)
    # keep-mask for the diagonal 128x128 block of S^T[k, q]: keep k <= q
    tri = np.triu(np.ones((128, 128), dtype=np.float32))
    in_maps = []
    for c in range(N_CORES):
        sl = slice(c * DHC, (c + 1) * DHC)
        in_maps.append(
            {
                "x": x,
                "wq": np.ascontiguousarray(np.asarray(W_q, np.float32)[:, sl]),
                "wk": np.ascontiguousarray(np.asarray(W_k, np.float32)[:, sl]),
                "wv": np.ascontiguousarray(np.asarray(W_v, np.float32)[:, sl]),
                "wo": np.ascontiguousarray(np.asarray(W_o, np.float32)[:, sl]),
                "tri": tri,
            }
        )
    nc = _build_nc()
    res = run_bass_kernel_spmd(nc, in_maps, core_ids=list(range(N_CORES)))
    out = np.concatenate([res.results[c]["out"] for c in range(N_CORES)], axis=1)
    return out.reshape(B, S, D)


# revision 19
# speedup vs baseline: 1.0876x; 1.0876x over previous
"""Multi-head causal attention (B=2, S=2048, D=1024, H=16) on 8 TRN2 NeuronCores.

Sharding: Megatron-style head parallelism. Core c owns heads {2c, 2c+1}:
  - W_q/W_k/W_v column slices [:, 128c:128(c+1)]  (2 heads x 64 dims)
  - attention for those heads over the full sequence (causal)
  - normalized context slices are AllGathered across cores (bf16, 4 chunks
    overlapped with attention of later tiles)
  - each core computes the output-projection column slice
    out[:, 128c:128(c+1)] = ctx_full @ W_o[:, 128c:128(c+1)]
  - host concatenates the 8 column slices (pure gather, no arithmetic)

Compute dtype: bf16 operands, fp32 PSUM accumulation. Scores are computed
transposed (S^T[k,q] = K Q^T) so the P^T tiles feed the A@V matmul directly;
softmax denominators come from an extra all-ones column appended to V.
"""

import numpy as np

import concourse.bass as bass
import concourse.mybir as mybir
from concourse import bacc, tile
from concourse.bass_utils import run_bass_kernel_spmd

N_CORES = 8
B, S, D = 2, 2048, 1024
H, DH = 16, 64
BS = B * S  # 4096
HPC = H // N_CORES  # heads per core = 2
DHC = HPC * DH  # 128 context dims per core
SCALE = 1.0 / 32.0  # 1/sqrt(D)
FP32 = mybir.dt.float32
BF16 = mybir.dt.bfloat16
Exp = mybir.ActivationFunctionType.Exp

NQ = 4  # q macro tiles of 512 per batch element
QM = S // NQ  # 512
NKT = S // 128  # 16 k-tiles of 128 per batch element
NCH = 4  # output chunks (b, m-pair); 1024 seq rows each

_nc_cache = {}


def _build():
    nc = bacc.Bacc(
        "TRN2", target_bir_lowering=False, debug=False, num_devices=N_CORES
    )

    x_d = nc.dram_tensor("x", [BS, D], FP32, kind="ExternalInput").ap()
    wq_d = nc.dram_tensor("wq", [D, DHC], FP32, kind="ExternalInput").ap()
    wk_d = nc.dram_tensor("wk", [D, DHC], FP32, kind="ExternalInput").ap()
    wv_d = nc.dram_tensor("wv", [D, DHC], FP32, kind="ExternalInput").ap()
    wo_d = nc.dram_tensor("wo", [D, DHC], FP32, kind="ExternalInput").ap()
    tri_d = nc.dram_tensor("tri", [128, 128], FP32, kind="ExternalInput").ap()
    out_d = nc.dram_tensor("out", [BS, DHC], FP32, kind="ExternalOutput").ap()

    with tile.TileContext(nc) as tc:
        with (
            tc.tile_pool(name="dram", bufs=1, space="DRAM") as dram,
            tc.tile_pool(name="pers", bufs=1) as pers,
            tc.tile_pool(name="ptp", bufs=4) as ptp,
            tc.tile_pool(name="nw", bufs=3) as nw,
            tc.tile_pool(name="ps_s", bufs=2, space="PSUM") as ps_s,
            tc.tile_pool(name="ps_c", bufs=2, space="PSUM") as ps_c,
            tc.tile_pool(name="ps_m", bufs=2, space="PSUM") as ps_m,
        ):
            # ---- persistent SBUF ----
            qt_sb = [pers.tile([128, S], BF16, name=f"qt{b}") for b in range(B)]
            kt_sb = [pers.tile([128, S], BF16, name=f"kt{b}") for b in range(B)]
            # V tiles: per k-tile layout [h0 64 | ones | h1 64 | ones] (130 cols)
            v_sb = [pers.tile([128, NKT * 130], BF16, name=f"v{b}") for b in range(B)]
            wq_sb = pers.tile([128, 8, DHC], BF16, name="wq_sb")
            wk_sb = pers.tile([128, 8, DHC], BF16, name="wk_sb")
            wv_sb = pers.tile([128, 8, DHC], BF16, name="wv_sb")
            wo_sb = pers.tile([128, 8, DHC], BF16, name="wo_sb")
            tri_sb = pers.tile([128, 128], BF16, name="tri_sb")

            # ---- weights: load fp32 (gpsimd queue), cast to bf16 ----
            wtmp = pers.tile([128, 8, DHC], FP32, name="wtmp")
            for w_d, w_sb in ((wq_d, wq_sb), (wk_d, wk_sb), (wv_d, wv_sb), (wo_d, wo_sb)):
                nc.gpsimd.dma_start(
                    wtmp[:], w_d.rearrange("(c p) n -> p c n", p=128)
                )
                nc.vector.tensor_copy(w_sb[:], wtmp[:])
            tri_f = nw.tile([128, 128], FP32, name="tri_f")
            nc.gpsimd.dma_start(tri_f[:], tri_d[:])
            nc.vector.tensor_copy(tri_sb[:], tri_f[:])

            # ---- attention output chunks (b, m-pair) for collective overlap ----
            ctx_in_c = [
                dram.tile([DHC, 2 * QM], BF16, name=f"ctx_in{k}") for k in range(NCH)
            ]
            ctx_all_c = [
                dram.tile(
                    [N_CORES * DHC, 2 * QM], BF16, name=f"ctx_all{k}",
                    addr_space="Shared",
                )
                for k in range(NCH)
            ]

            def attention(b, m):
                qcols = slice(m * QM, (m + 1) * QM)
                ctx_ps = [
                    ps_c.tile([65, QM], FP32, name=f"ctx_ps{h}", tag="c")
                    for h in range(HPC)
                ]
                n_kt = 4 * m + 4
                for kt in range(n_kt):
                    s_ps = ps_s.tile([128, 2 * QM], FP32, name="s_ps", tag="s")
                    for h in range(HPC):
                        nc.tensor.matmul(
                            s_ps[:, h * QM : (h + 1) * QM],
                            kt_sb[b][h * 64 : (h + 1) * 64, kt * 128 : (kt + 1) * 128],
                            qt_sb[b][h * 64 : (h + 1) * 64, qcols],
                            start=True,
                            stop=True,
                            tile_position=(h * 64, 0),
                        )
                    pt = ptp.tile([128, 2 * QM], BF16, name="pt")
                    j = kt - 4 * m  # diagonal block index if >= 0
                    if j < 0:
                        nc.scalar.activation(pt[:], s_ps[:], Exp, scale=SCALE)
                    else:
                        qs = 128 * j
                        for h in range(HPC):
                            nc.scalar.activation(
                                pt[:, h * QM + qs : (h + 1) * QM],
                                s_ps[:, h * QM + qs : (h + 1) * QM],
                                Exp,
                                scale=SCALE,
                            )
                            nc.vector.tensor_mul(
                                pt[:, h * QM + qs : h * QM + qs + 128],
                                pt[:, h * QM + qs : h * QM + qs + 128],
                                tri_sb[:],
                            )
                    qs = max(0, 128 * (kt - 4 * m))
                    for h in range(HPC):
                        nc.tensor.matmul(
                            ctx_ps[h][:, qs:QM],
                            v_sb[b][:, kt * 130 + h * 65 : kt * 130 + (h + 1) * 65],
                            pt[:, h * QM + qs : (h + 1) * QM],
                            start=(kt == 0),
                            stop=(kt == n_kt - 1),
                        )
                # normalize: ctx[0:64] * (1 / rowsum); rowsum in row 64
                k = b * 2 + m // 2
                for h in range(HPC):
                    recip = nw.tile([1, QM], FP32, name="recip")
                    nc.vector.reciprocal(recip[:], ctx_ps[h][64:65, :])
                    bcast = nw.tile([64, QM], FP32, name="bcast")
                    nc.gpsimd.partition_broadcast(bcast[:], recip[:])
                    ctxn = nw.tile([64, QM], BF16, name="ctxn")
                    nc.vector.tensor_mul(ctxn[:], ctx_ps[h][0:64, :], bcast[:])
                    nc.gpsimd.dma_start(
                        ctx_in_c[k][
                            h * 64 : (h + 1) * 64,
                            (m % 2) * QM : (m % 2 + 1) * QM,
                        ],
                        ctxn[:],
                    )

            def allgather(k):
                nc.gpsimd.collective_compute(
                    "AllGather",
                    mybir.AluOpType.bypass,
                    replica_groups=[list(range(N_CORES))],
                    ins=[ctx_in_c[k][:]],
                    outs=[ctx_all_c[k][:]],
                )

            def qkv(b, xt_sb):
                nc.gpsimd.memset(v_sb[b][:], 1.0)
                for j in range(NQ):
                    cols = slice(b * S + j * QM, b * S + (j + 1) * QM)
                    for w_sb, t_sb in ((wq_sb, qt_sb[b]), (wk_sb, kt_sb[b])):
                        ps = ps_m.tile([128, QM], FP32, name="ps_qk", tag="m")
                        for dt in range(8):
                            nc.tensor.matmul(
                                ps[:],
                                w_sb[:, dt, :],
                                xt_sb[:, dt, cols],
                                start=(dt == 0),
                                stop=(dt == 7),
                            )
                        nc.vector.tensor_copy(t_sb[:, j * QM : (j + 1) * QM], ps[:])
                    for st2 in range(4):
                        kt_i = j * 4 + st2
                        scol = slice(b * S + kt_i * 128, b * S + (kt_i + 1) * 128)
                        ps_v = ps_m.tile([128, QM], FP32, name="ps_v", tag="m")
                        for dt in range(8):
                            nc.tensor.matmul(
                                ps_v[:, 0:DHC],
                                xt_sb[:, dt, scol],
                                wv_sb[:, dt, :],
                                start=(dt == 0),
                                stop=(dt == 7),
                            )
                        dst = v_sb[b][:, kt_i * 130 : kt_i * 130 + 130].rearrange(
                            "p (g c) -> p g c", g=2
                        )[:, :, 0:64]
                        src = ps_v[:, 0:DHC].rearrange("p (g c) -> p g c", g=2)
                        nc.vector.tensor_copy(dst, src)

            def outproj(k, cfp):
                cf = cfp.tile([128, 8, 2 * QM], BF16, name="cf", tag="cf", bufs=2)
                for dt in range(8):
                    nc.sync.dma_start(
                        cf[:, dt, :], ctx_all_c[k][dt * 128 : (dt + 1) * 128, :]
                    )
                o_sb = nw.tile([128, 8, DHC], FP32, name="o_sb", tag="o", bufs=2)
                for qi in range(8):
                    ps_o = ps_m.tile([128, QM], FP32, name="ps_o", tag="m")
                    for dt in range(8):
                        nc.tensor.matmul(
                            ps_o[:, 0:DHC],
                            cf[:, dt, qi * 128 : (qi + 1) * 128],
                            wo_sb[:, dt, :],
                            start=(dt == 0),
                            stop=(dt == 7),
                        )
                    eng = nc.scalar if qi % 2 == 0 else nc.vector
                    if eng is nc.scalar:
                        eng.copy(o_sb[:, qi, :], ps_o[:, 0:DHC])
                    else:
                        eng.tensor_copy(o_sb[:, qi, :], ps_o[:, 0:DHC])
                nc.gpsimd.dma_start(
                    out_d[k * 1024 : (k + 1) * 1024, :].rearrange(
                        "(c p) n -> p c n", p=128
                    ),
                    o_sb[:],
                )

            # ---- x: load fp32, cast bf16, write back, transpose-read ----
            xbf_dram = dram.tile([BS, D], BF16, name="xbf_dram")
            with (
                tc.tile_pool(name="xtp", bufs=1) as xtp,
                tc.tile_pool(name="ldx", bufs=2) as ldx,
            ):
                xt_sb = xtp.tile([128, 8, BS], BF16, name="xt_sb")
                for g in range(8):
                    rows = slice(g * 512, (g + 1) * 512)
                    x_f = ldx.tile([128, 4, D], FP32, name="x_f", tag="xf")
                    nc.sync.dma_start(
                        x_f[:], x_d[rows, :].rearrange("(c p) d -> p c d", p=128)
                    )
                    x_b = ldx.tile([128, 4, D], BF16, name="x_b", tag="xb")
                    if g % 2 == 0:
                        nc.scalar.copy(x_b[:], x_f[:])
                    else:
                        nc.vector.tensor_copy(x_b[:], x_f[:])
                    nc.sync.dma_start(
                        xbf_dram[rows, :].rearrange("(c p) d -> p c d", p=128), x_b[:]
                    )
                for b in range(B):
                    for dt in range(8):
                        nc.sync.dma_start_transpose(
                            xt_sb[:, dt, b * S : (b + 1) * S],
                            xbf_dram[b * S : (b + 1) * S, dt * 128 : (dt + 1) * 128],
                        )
                    qkv(b, xt_sb)

            # ---- attention + chunked collective + output projection ----
            with tc.tile_pool(name="cfp", bufs=1) as cfp:
                attention(0, 0)
                attention(0, 1)
                allgather(0)
                attention(0, 2)
                attention(0, 3)
                allgather(1)
                outproj(0, cfp)
                attention(1, 0)
                attention(1, 1)
                allgather(2)
                outproj(1, cfp)
                attention(1, 2)
                attention(1, 3)
                allgather(3)
                outproj(2, cfp)
                outproj(3, cfp)

    nc.compile()
    return nc


def _build_nc():
    if "nc" not in _nc_cache:
        _nc_cache["nc"] = _build()
    return _nc_cache["nc"]


def kernel(x, W_q, W_k, W_v, W_o):
    x = np.ascontiguousarray(np.asarray(x, dtype=np.float32)).reshape(BS, D)
    # keep-mask for the diagonal 128x128 block of S^T[k, q]: keep k <= q
    tri = np.triu(np.ones((128, 128), dtype=np.float32))
    in_maps = []
    for c in range(N_CORES):
        sl = slice(c * DHC, (c + 1) * DHC)
        in_maps.append(
            {
                "x": x,
                "wq": np.ascontiguousarray(np.asarray(W_q, np.float32)[:, sl]),
                "wk": np.ascontiguousarray(np.asarray(W_k, np.float32)[:, sl]),
                "wv": np.ascontiguousarray(np.asarray(W_v, np.float32)[:, sl]),
                "wo": np.ascontiguousarray(np.asarray(W_o, np.float32)[:, sl]),
                "tri": tri,
            }
        )
    nc = _build_nc()
    res = run_bass_kernel_spmd(nc, in_maps, core_ids=list(range(N_CORES)))
    out = np.concatenate([res.results[c]["out"] for c in range(N_CORES)], axis=1)
    return out.reshape(B, S, D)


# revision 23
# speedup vs baseline: 1.1009x; 1.0122x over previous
"""Multi-head causal attention (B=2, S=2048, D=1024, H=16) on 8 TRN2 NeuronCores.

Sharding: Megatron-style head parallelism. Core c owns heads {2c, 2c+1}:
  - W_q/W_k/W_v column slices [:, 128c:128(c+1)]  (2 heads x 64 dims)
  - attention for those heads over the full sequence (causal)
  - normalized context slices are AllGathered across cores (bf16, 4 chunks
    overlapped with attention of later tiles)
  - each core computes the output-projection column slice
    out[:, 128c:128(c+1)] = ctx_full @ W_o[:, 128c:128(c+1)]
  - host concatenates the 8 column slices (pure gather, no arithmetic)

Compute dtype: bf16 operands, fp32 PSUM accumulation. Scores are computed
transposed (S^T[k,q] = K Q^T) so the P^T tiles feed the A@V matmul directly;
softmax denominators come from an extra all-ones column appended to V.
Pipelining: x is cast+transposed per 512-row chunk with QKV projections and
the first attention tiles interleaved, so the TensorEngine never sits idle
behind the DMA pipeline.
"""

import numpy as np

import concourse.bass as bass
import concourse.mybir as mybir
from concourse import bacc, tile
from concourse.bass_utils import run_bass_kernel_spmd

N_CORES = 8
B, S, D = 2, 2048, 1024
H, DH = 16, 64
BS = B * S  # 4096
HPC = H // N_CORES  # heads per core = 2
DHC = HPC * DH  # 128 context dims per core
SCALE = 1.0 / 32.0  # 1/sqrt(D)
FP32 = mybir.dt.float32
BF16 = mybir.dt.bfloat16
Exp = mybir.ActivationFunctionType.Exp

NQ = 4  # q macro tiles of 512 per batch element
QM = S // NQ  # 512
NKT = S // 128  # 16 k-tiles of 128 per batch element

_nc_cache = {}


def _build():
    nc = bacc.Bacc(
        "TRN2", target_bir_lowering=False, debug=False, num_devices=N_CORES
    )

    x_d = nc.dram_tensor("x", [BS, D], FP32, kind="ExternalInput").ap()
    wq_d = nc.dram_tensor("wq", [D, DHC], FP32, kind="ExternalInput").ap()
    wk_d = nc.dram_tensor("wk", [D, DHC], FP32, kind="ExternalInput").ap()
    wv_d = nc.dram_tensor("wv", [D, DHC], FP32, kind="ExternalInput").ap()
    wo_d = nc.dram_tensor("wo", [D, DHC], FP32, kind="ExternalInput").ap()
    tri_d = nc.dram_tensor("tri", [128, 128], FP32, kind="ExternalInput").ap()
    out_d = nc.dram_tensor("out", [BS, DHC], FP32, kind="ExternalOutput").ap()

    with tile.TileContext(nc) as tc:
        with (
            tc.tile_pool(name="dram", bufs=1, space="DRAM") as dram,
            tc.tile_pool(name="pers", bufs=1) as pers,
            tc.tile_pool(name="ptp", bufs=4) as ptp,
            tc.tile_pool(name="nw", bufs=3) as nw,
            tc.tile_pool(name="ps_s", bufs=2, space="PSUM") as ps_s,
            tc.tile_pool(name="ps_c", bufs=2, space="PSUM") as ps_c,
            tc.tile_pool(name="ps_m", bufs=2, space="PSUM") as ps_m,
        ):
            # ---- persistent SBUF ----
            qt_sb = [pers.tile([128, S], BF16, name=f"qt{b}") for b in range(B)]
            kt_sb = [pers.tile([128, S], BF16, name=f"kt{b}") for b in range(B)]
            # V tiles: per k-tile layout [h0 64 | ones | h1 64 | ones] (130 cols)
            v_sb = [pers.tile([128, NKT * 130], BF16, name=f"v{b}") for b in range(B)]
            wq_sb = pers.tile([128, 8, DHC], BF16, name="wq_sb")
            wk_sb = pers.tile([128, 8, DHC], BF16, name="wk_sb")
            wv_sb = pers.tile([128, 8, DHC], BF16, name="wv_sb")
            wo_sb = pers.tile([128, 8, DHC], BF16, name="wo_sb")
            tri_sb = pers.tile([128, 128], BF16, name="tri_sb")

            # ---- weights: load fp32 (gpsimd queue), cast to bf16 ----
            wtmp = pers.tile([128, 8, DHC], FP32, name="wtmp")
            for w_d, w_sb in ((wq_d, wq_sb), (wk_d, wk_sb), (wv_d, wv_sb), (wo_d, wo_sb)):
                nc.gpsimd.dma_start(
                    wtmp[:], w_d.rearrange("(c p) n -> p c n", p=128)
                )
                nc.vector.tensor_copy(w_sb[:], wtmp[:])
            tri_f = nw.tile([128, 128], FP32, name="tri_f")
            nc.gpsimd.dma_start(tri_f[:], tri_d[:])
            nc.vector.tensor_copy(tri_sb[:], tri_f[:])

            # ---- attention output chunks (one per m tile, both b) ----
            ctx_in_c = [
                dram.tile([DHC, 2 * QM], BF16, name=f"ctx_in{k}") for k in range(NQ)
            ]
            ctx_all_c = [
                dram.tile(
                    [N_CORES * DHC, 2 * QM], BF16, name=f"ctx_all{k}",
                    addr_space="Shared",
                )
                for k in range(NQ)
            ]

            def attention(b, m):
                qcols = slice(m * QM, (m + 1) * QM)
                ctx_ps = [
                    ps_c.tile([65, QM], FP32, name=f"ctx_ps{h}", tag="c")
                    for h in range(HPC)
                ]
                n_kt = 4 * m + 4
                for kt in range(n_kt):
                    s_ps = ps_s.tile([128, 2 * QM], FP32, name="s_ps", tag="s")
                    for h in range(HPC):
                        nc.tensor.matmul(
                            s_ps[:, h * QM : (h + 1) * QM],
                            kt_sb[b][h * 64 : (h + 1) * 64, kt * 128 : (kt + 1) * 128],
                            qt_sb[b][h * 64 : (h + 1) * 64, qcols],
                            start=True,
                            stop=True,
                            tile_position=(h * 64, 0),
                        )
                    pt = ptp.tile([128, 2 * QM], BF16, name="pt")
                    j = kt - 4 * m  # diagonal block index if >= 0
                    if j < 0:
                        nc.scalar.activation(pt[:], s_ps[:], Exp, scale=SCALE)
                    else:
                        qs = 128 * j
                        for h in range(HPC):
                            nc.scalar.activation(
                                pt[:, h * QM + qs : (h + 1) * QM],
                                s_ps[:, h * QM + qs : (h + 1) * QM],
                                Exp,
                                scale=SCALE,
                            )
                            nc.vector.tensor_mul(
                                pt[:, h * QM + qs : h * QM + qs + 128],
                                pt[:, h * QM + qs : h * QM + qs + 128],
                                tri_sb[:],
                            )
                    qs = max(0, 128 * (kt - 4 * m))
                    for h in range(HPC):
                        nc.tensor.matmul(
                            ctx_ps[h][:, qs:QM],
                            v_sb[b][:, kt * 130 + h * 65 : kt * 130 + (h + 1) * 65],
                            pt[:, h * QM + qs : (h + 1) * QM],
                            start=(kt == 0),
                            stop=(kt == n_kt - 1),
                        )
                # normalize: ctx[0:64] * (1 / rowsum); rowsum in row 64
                for h in range(HPC):
                    recip = nw.tile([1, QM], FP32, name="recip")
                    nc.vector.reciprocal(recip[:], ctx_ps[h][64:65, :])
                    bcast = nw.tile([64, QM], FP32, name="bcast")
                    nc.gpsimd.partition_broadcast(bcast[:], recip[:])
                    ctxn = nw.tile([64, QM], BF16, name="ctxn")
                    nc.vector.tensor_mul(ctxn[:], ctx_ps[h][0:64, :], bcast[:])
                    nc.gpsimd.dma_start(
                        ctx_in_c[m][h * 64 : (h + 1) * 64, b * QM : (b + 1) * QM],
                        ctxn[:],
                    )

            def allgather(m):
                nc.gpsimd.collective_compute(
                    "AllGather",
                    mybir.AluOpType.bypass,
                    replica_groups=[list(range(N_CORES))],
                    ins=[ctx_in_c[m][:]],
                    outs=[ctx_all_c[m][:]],
                )

            def qkv_chunk(b, j, xt_sb):
                cols = slice(b * S + j * QM, b * S + (j + 1) * QM)
                for w_sb, t_sb in ((wq_sb, qt_sb[b]), (wk_sb, kt_sb[b])):
                    ps = ps_m.tile([128, QM], FP32, name="ps_qk", tag="m")
                    for dt in range(8):
                        nc.tensor.matmul(
                            ps[:],
                            w_sb[:, dt, :],
                            xt_sb[:, dt, cols],
                            start=(dt == 0),
                            stop=(dt == 7),
                        )
                    nc.vector.tensor_copy(t_sb[:, j * QM : (j + 1) * QM], ps[:])
                for st2 in range(4):
                    kt_i = j * 4 + st2
                    scol = slice(b * S + kt_i * 128, b * S + (kt_i + 1) * 128)
                    ps_v = ps_m.tile([128, QM], FP32, name="ps_v", tag="m")
                    for dt in range(8):
                        nc.tensor.matmul(
                            ps_v[:, 0:DHC],
                            xt_sb[:, dt, scol],
                            wv_sb[:, dt, :],
                            start=(dt == 0),
                            stop=(dt == 7),
                        )
                    dst = v_sb[b][:, kt_i * 130 : kt_i * 130 + 130].rearrange(
                        "p (g c) -> p g c", g=2
                    )[:, :, 0:64]
                    src = ps_v[:, 0:DHC].rearrange("p (g c) -> p g c", g=2)
                    nc.vector.tensor_copy(dst, src)

            def outproj(m, cfp):
                cf = cfp.tile([128, 8, 2 * QM], BF16, name="cf", tag="cf", bufs=2)
                for dt in range(8):
                    nc.sync.dma_start(
                        cf[:, dt, :], ctx_all_c[m][dt * 128 : (dt + 1) * 128, :]
                    )
                o_sb = nw.tile([128, 8, DHC], FP32, name="o_sb", tag="o", bufs=2)
                for bb in range(B):
                    for qi in range(4):
                        ps_o = ps_m.tile([128, QM], FP32, name="ps_o", tag="m")
                        for dt in range(8):
                            nc.tensor.matmul(
                                ps_o[:, 0:DHC],
                                cf[:, dt, bb * QM + qi * 128 : bb * QM + (qi + 1) * 128],
                                wo_sb[:, dt, :],
                                start=(dt == 0),
                                stop=(dt == 7),
                            )
                        sl = bb * 4 + qi
                        if sl % 2 == 0:
                            nc.scalar.copy(o_sb[:, sl, :], ps_o[:, 0:DHC])
                        else:
                            nc.vector.tensor_copy(o_sb[:, sl, :], ps_o[:, 0:DHC])
                for bb in range(B):
                    nc.gpsimd.dma_start(
                        out_d[bb * S + m * QM : bb * S + (m + 1) * QM, :].rearrange(
                            "(c p) n -> p c n", p=128
                        ),
                        o_sb[:, bb * 4 : (bb + 1) * 4, :],
                    )

            # ---- x: load fp32, cast bf16, write back, transpose-read ----
            # group g covers seq rows [g*512, (g+1)*512) = (b = g//4, j = g%4).
            # b-alternating order so attention on b0/b1 m=0 can start early.
            xbf_dram = dram.tile([BS, D], BF16, name="xbf_dram")
            with (
                tc.tile_pool(name="xtp", bufs=1) as xtp,
                tc.tile_pool(name="ldx", bufs=2) as ldx,
            ):
                xt_sb = xtp.tile([128, 8, BS], BF16, name="xt_sb")
                memset_done = set()
                for gi, g in enumerate([0, 4, 1, 5, 2, 6, 3, 7]):
                    b, j = g // 4, g % 4
                    rows = slice(g * 512, (g + 1) * 512)
                    x_f = ldx.tile([128, 4, D], FP32, name="x_f", tag="xf")
                    nc.sync.dma_start(
                        x_f[:], x_d[rows, :].rearrange("(c p) d -> p c d", p=128)
                    )
                    x_b = ldx.tile([128, 4, D], BF16, name="x_b", tag="xb")
                    if gi % 2 == 0:
                        nc.vector.tensor_copy(x_b[:], x_f[:])
                    else:
                        nc.scalar.copy(x_b[:], x_f[:])
                    nc.sync.dma_start(
                        xbf_dram[rows, :].rearrange("(c p) d -> p c d", p=128), x_b[:]
                    )
                    # transposed read of this 512-row chunk (8 column tiles)
                    for dt in range(8):
                        nc.sync.dma_start_transpose(
                            xt_sb[:, dt, g * 512 : (g + 1) * 512],
                            xbf_dram[rows, dt * 128 : (dt + 1) * 128],
                        )
                    if b not in memset_done:
                        memset_done.add(b)
                        nc.gpsimd.memset(v_sb[b][:], 1.0)
                    qkv_chunk(b, j, xt_sb)

            # ---- attention + chunked collective + output projection ----
            with tc.tile_pool(name="cfp", bufs=1) as cfp:
                attention(0, 0)
                attention(1, 0)
                allgather(0)
                attention(0, 1)
                attention(1, 1)
                allgather(1)
                attention(0, 2)
                attention(1, 2)
                allgather(2)
                outproj(0, cfp)
                attention(0, 3)
                outproj(1, cfp)
                attention(1, 3)
                allgather(3)
                outproj(2, cfp)
                outproj(3, cfp)

    nc.compile()
    return nc


def _build_nc():
    if "nc" not in _nc_cache:
        _nc_cache["nc"] = _build()
    return _nc_cache["nc"]


def kernel(x, W_q, W_k, W_v, W_o):
    x = np.ascontiguousarray(np.asarray(x, dtype=np.float32)).reshape(BS, D)
    # keep-mask for the diagonal 128x128 block of S^T[k, q]: keep k <= q
    tri = np.triu(np.ones((128, 128), dtype=np.float32))
    in_maps = []
    for c in range(N_CORES):
        sl = slice(c * DHC, (c + 1) * DHC)
        in_maps.append(
            {
                "x": x,
                "wq": np.ascontiguousarray(np.asarray(W_q, np.float32)[:, sl]),
                "wk": np.ascontiguousarray(np.asarray(W_k, np.float32)[:, sl]),
                "wv": np.ascontiguousarray(np.asarray(W_v, np.float32)[:, sl]),
                "wo": np.ascontiguousarray(np.asarray(W_o, np.float32)[:, sl]),
                "tri": tri,
            }
        )
    nc = _build_nc()
    res = run_bass_kernel_spmd(nc, in_maps, core_ids=list(range(N_CORES)))
    out = np.concatenate([res.results[c]["out"] for c in range(N_CORES)], axis=1)
    return out.reshape(B, S, D)


# revision 26
# speedup vs baseline: 1.1381x; 1.0338x over previous
"""Multi-head causal attention (B=2, S=2048, D=1024, H=16) on 8 TRN2 NeuronCores.

Sharding: Megatron-style head parallelism. Core c owns heads {2c, 2c+1}:
  - W_q/W_k/W_v column slices [:, 128c:128(c+1)]  (2 heads x 64 dims)
  - attention for those heads over the full sequence (causal)
  - normalized context slices are AllGathered across cores (bf16, 4 chunks
    overlapped with attention of later tiles)
  - each core computes the output-projection column slice
    out[:, 128c:128(c+1)] = ctx_full @ W_o[:, 128c:128(c+1)]
  - host concatenates the 8 column slices (pure gather, no arithmetic)

Compute dtype: bf16 operands, fp32 PSUM accumulation. Scores are computed
transposed (S^T[k,q] = K Q^T) so the P^T tiles feed the A@V matmul directly;
softmax denominators come from an extra all-ones column appended to V.
Pipelining: x is cast+transposed per 512-row chunk with QKV projections and
the first attention tiles interleaved, so the TensorEngine never sits idle
behind the DMA pipeline.
"""

import numpy as np

import concourse.bass as bass
import concourse.mybir as mybir
from concourse import bacc, tile
from concourse.bass_utils import run_bass_kernel_spmd

N_CORES = 8
B, S, D = 2, 2048, 1024
H, DH = 16, 64
BS = B * S  # 4096
HPC = H // N_CORES  # heads per core = 2
DHC = HPC * DH  # 128 context dims per core
SCALE = 1.0 / 32.0  # 1/sqrt(D)
FP32 = mybir.dt.float32
BF16 = mybir.dt.bfloat16
Exp = mybir.ActivationFunctionType.Exp

NQ = 4  # q macro tiles of 512 per batch element
QM = S // NQ  # 512
NKT = S // 128  # 16 k-tiles of 128 per batch element

_nc_cache = {}


def _build():
    nc = bacc.Bacc(
        "TRN2", target_bir_lowering=False, debug=False, num_devices=N_CORES
    )

    x_d = nc.dram_tensor("x", [BS, D], FP32, kind="ExternalInput").ap()
    wq_d = nc.dram_tensor("wq", [D, DHC], FP32, kind="ExternalInput").ap()
    wk_d = nc.dram_tensor("wk", [D, DHC], FP32, kind="ExternalInput").ap()
    wv_d = nc.dram_tensor("wv", [D, DHC], FP32, kind="ExternalInput").ap()
    wo_d = nc.dram_tensor("wo", [D, DHC], FP32, kind="ExternalInput").ap()
    tri_d = nc.dram_tensor("tri", [128, 128], FP32, kind="ExternalInput").ap()
    out_d = nc.dram_tensor("out", [BS, DHC], FP32, kind="ExternalOutput").ap()

    with tile.TileContext(nc) as tc:
        with (
            tc.tile_pool(name="dram", bufs=1, space="DRAM") as dram,
            tc.tile_pool(name="pers", bufs=1) as pers,
            tc.tile_pool(name="ptp", bufs=4) as ptp,
            tc.tile_pool(name="nw", bufs=3) as nw,
            tc.tile_pool(name="ps_s", bufs=2, space="PSUM") as ps_s,
            tc.tile_pool(name="ps_c", bufs=2, space="PSUM") as ps_c,
            tc.tile_pool(name="ps_m", bufs=2, space="PSUM") as ps_m,
        ):
            # ---- persistent SBUF ----
            qt_sb = [pers.tile([128, S], BF16, name=f"qt{b}") for b in range(B)]
            kt_sb = [pers.tile([128, S], BF16, name=f"kt{b}") for b in range(B)]
            # V tiles: per k-tile layout [h0 64 | ones | h1 64 | ones] (130 cols)
            v_sb = [pers.tile([128, NKT * 130], BF16, name=f"v{b}") for b in range(B)]
            wq_sb = pers.tile([128, 8, DHC], BF16, name="wq_sb")
            wk_sb = pers.tile([128, 8, DHC], BF16, name="wk_sb")
            wv_sb = pers.tile([128, 8, DHC], BF16, name="wv_sb")
            wo_sb = pers.tile([128, 8, DHC], BF16, name="wo_sb")
            tri_sb = pers.tile([128, 128], BF16, name="tri_sb")
            ones_sb = pers.tile([1, 64], BF16, name="ones_sb")
            nc.vector.memset(ones_sb[:], 1.0)

            # ---- weights: load fp32 (gpsimd queue), cast to bf16 ----
            wtmp = pers.tile([128, 8, DHC], FP32, name="wtmp")
            for w_d, w_sb in ((wq_d, wq_sb), (wk_d, wk_sb), (wv_d, wv_sb), (wo_d, wo_sb)):
                nc.gpsimd.dma_start(
                    wtmp[:], w_d.rearrange("(c p) n -> p c n", p=128)
                )
                nc.vector.tensor_copy(w_sb[:], wtmp[:])
            tri_f = nw.tile([128, 128], FP32, name="tri_f")
            nc.gpsimd.dma_start(tri_f[:], tri_d[:])
            nc.vector.tensor_copy(tri_sb[:], tri_f[:])

            # ---- attention output chunks (one per m tile, both b) ----
            ctx_in_c = [
                dram.tile([DHC, 2 * QM], BF16, name=f"ctx_in{k}") for k in range(NQ)
            ]
            ctx_all_c = [
                dram.tile(
                    [N_CORES * DHC, 2 * QM], BF16, name=f"ctx_all{k}",
                    addr_space="Shared",
                )
                for k in range(NQ)
            ]

            def attention(b, m):
                qcols = slice(m * QM, (m + 1) * QM)
                ctx_ps = [
                    ps_c.tile([65, QM], FP32, name=f"ctx_ps{h}", tag="c")
                    for h in range(HPC)
                ]
                n_kt = 4 * m + 4
                for kt in range(n_kt):
                    s_ps = ps_s.tile([128, 2 * QM], FP32, name="s_ps", tag="s")
                    for h in range(HPC):
                        nc.tensor.matmul(
                            s_ps[:, h * QM : (h + 1) * QM],
                            kt_sb[b][h * 64 : (h + 1) * 64, kt * 128 : (kt + 1) * 128],
                            qt_sb[b][h * 64 : (h + 1) * 64, qcols],
                            start=True,
                            stop=True,
                            tile_position=(h * 64, 0),
                        )
                    pt = ptp.tile([128, 2 * QM], BF16, name="pt")
                    j = kt - 4 * m  # diagonal block index if >= 0
                    if j < 0:
                        nc.scalar.activation(pt[:], s_ps[:], Exp, scale=SCALE)
                    else:
                        qs = 128 * j
                        for h in range(HPC):
                            nc.scalar.activation(
                                pt[:, h * QM + qs : (h + 1) * QM],
                                s_ps[:, h * QM + qs : (h + 1) * QM],
                                Exp,
                                scale=SCALE,
                            )
                            nc.vector.tensor_mul(
                                pt[:, h * QM + qs : h * QM + qs + 128],
                                pt[:, h * QM + qs : h * QM + qs + 128],
                                tri_sb[:],
                            )
                    qs = max(0, 128 * (kt - 4 * m))
                    for h in range(HPC):
                        nc.tensor.matmul(
                            ctx_ps[h][:, qs:QM],
                            v_sb[b][:, kt * 130 + h * 65 : kt * 130 + (h + 1) * 65],
                            pt[:, h * QM + qs : (h + 1) * QM],
                            start=(kt == 0),
                            stop=(kt == n_kt - 1),
                        )
                # normalize: ctx[0:64] * (1 / rowsum); rowsum in row 64
                for h in range(HPC):
                    recip = nw.tile([1, QM], BF16, name="recip")
                    with nc.allow_low_precision(reason="softmax denom to bf16"):
                        nc.vector.reciprocal(recip[:], ctx_ps[h][64:65, :])
                    bc_ps = ps_m.tile([128, QM], FP32, name="bc_ps", tag="m")
                    nc.tensor.matmul(
                        bc_ps[0:64, :], ones_sb[:], recip[:], start=True, stop=True
                    )
                    bc_sb = nw.tile([64, QM], FP32, name="bc_sb")
                    nc.vector.tensor_copy(bc_sb[:], bc_ps[0:64, :])
                    ctxn = nw.tile([64, QM], BF16, name="ctxn")
                    nc.vector.tensor_mul(ctxn[:], ctx_ps[h][0:64, :], bc_sb[:])
                    nc.scalar.dma_start(
                        ctx_in_c[m][h * 64 : (h + 1) * 64, b * QM : (b + 1) * QM],
                        ctxn[:],
                    )

            def allgather(m):
                nc.gpsimd.collective_compute(
                    "AllGather",
                    mybir.AluOpType.bypass,
                    replica_groups=[list(range(N_CORES))],
                    ins=[ctx_in_c[m][:]],
                    outs=[ctx_all_c[m][:]],
                )

            def qkv_chunk(b, j, xt_sb):
                cols = slice(b * S + j * QM, b * S + (j + 1) * QM)
                for w_sb, t_sb in ((wq_sb, qt_sb[b]), (wk_sb, kt_sb[b])):
                    ps = ps_m.tile([128, QM], FP32, name="ps_qk", tag="m")
                    for dt in range(8):
                        nc.tensor.matmul(
                            ps[:],
                            w_sb[:, dt, :],
                            xt_sb[:, dt, cols],
                            start=(dt == 0),
                            stop=(dt == 7),
                        )
                    nc.vector.tensor_copy(t_sb[:, j * QM : (j + 1) * QM], ps[:])
                for st2 in range(4):
                    kt_i = j * 4 + st2
                    scol = slice(b * S + kt_i * 128, b * S + (kt_i + 1) * 128)
                    ps_v = ps_m.tile([128, QM], FP32, name="ps_v", tag="m")
                    for dt in range(8):
                        nc.tensor.matmul(
                            ps_v[:, 0:DHC],
                            xt_sb[:, dt, scol],
                            wv_sb[:, dt, :],
                            start=(dt == 0),
                            stop=(dt == 7),
                        )
                    dst = v_sb[b][:, kt_i * 130 : kt_i * 130 + 130].rearrange(
                        "p (g c) -> p g c", g=2
                    )[:, :, 0:64]
                    src = ps_v[:, 0:DHC].rearrange("p (g c) -> p g c", g=2)
                    nc.vector.tensor_copy(dst, src)

            def outproj(m, cfp):
                cf = cfp.tile([128, 8, 2 * QM], BF16, name="cf", tag="cf", bufs=2)
                for dt in range(8):
                    nc.sync.dma_start(
                        cf[:, dt, :], ctx_all_c[m][dt * 128 : (dt + 1) * 128, :]
                    )
                o_sb = nw.tile([128, 8, DHC], FP32, name="o_sb", tag="o", bufs=2)
                for bb in range(B):
                    for qi in range(4):
                        ps_o = ps_m.tile([128, QM], FP32, name="ps_o", tag="m")
                        for dt in range(8):
                            nc.tensor.matmul(
                                ps_o[:, 0:DHC],
                                cf[:, dt, bb * QM + qi * 128 : bb * QM + (qi + 1) * 128],
                                wo_sb[:, dt, :],
                                start=(dt == 0),
                                stop=(dt == 7),
                            )
                        sl = bb * 4 + qi
                        if sl % 2 == 0:
                            nc.scalar.copy(o_sb[:, sl, :], ps_o[:, 0:DHC])
                        else:
                            nc.vector.tensor_copy(o_sb[:, sl, :], ps_o[:, 0:DHC])
                for bb in range(B):
                    nc.sync.dma_start(
                        out_d[bb * S + m * QM : bb * S + (m + 1) * QM, :].rearrange(
                            "(c p) n -> p c n", p=128
                        ),
                        o_sb[:, bb * 4 : (bb + 1) * 4, :],
                    )

            # ---- x: load fp32, cast bf16, write back, transpose-read ----
            # group g covers seq rows [g*512, (g+1)*512) = (b = g//4, j = g%4).
            # b-alternating order so attention on b0/b1 m=0 can start early.
            xbf_dram = dram.tile([BS, D], BF16, name="xbf_dram")
            with (
                tc.tile_pool(name="xtp", bufs=1) as xtp,
                tc.tile_pool(name="ldx", bufs=2) as ldx,
            ):
                xt_sb = xtp.tile([128, 8, BS], BF16, name="xt_sb")
                memset_done = set()
                for gi, g in enumerate([0, 4, 1, 5, 2, 6, 3, 7]):
                    b, j = g // 4, g % 4
                    rows = slice(g * 512, (g + 1) * 512)
                    x_f = ldx.tile([128, 4, D], FP32, name="x_f", tag="xf")
                    nc.sync.dma_start(
                        x_f[:], x_d[rows, :].rearrange("(c p) d -> p c d", p=128)
                    )
                    x_b = ldx.tile([128, 4, D], BF16, name="x_b", tag="xb")
                    if gi % 2 == 0:
                        nc.vector.tensor_copy(x_b[:], x_f[:])
                    else:
                        nc.scalar.copy(x_b[:], x_f[:])
                    nc.sync.dma_start(
                        xbf_dram[rows, :].rearrange("(c p) d -> p c d", p=128), x_b[:]
                    )
                    # transposed read of this 512-row chunk (8 column tiles)
                    for dt in range(8):
                        nc.sync.dma_start_transpose(
                            xt_sb[:, dt, g * 512 : (g + 1) * 512],
                            xbf_dram[rows, dt * 128 : (dt + 1) * 128],
                        )
                    if b not in memset_done:
                        memset_done.add(b)
                        nc.gpsimd.memset(v_sb[b][:], 1.0)
                    qkv_chunk(b, j, xt_sb)

            # ---- attention + chunked collective + output projection ----
            with tc.tile_pool(name="cfp", bufs=1) as cfp:
                attention(0, 0)
                attention(1, 0)
                allgather(0)
                attention(0, 1)
                attention(1, 1)
                allgather(1)
                attention(0, 2)
                attention(1, 2)
                allgather(2)
                outproj(0, cfp)
                attention(0, 3)
                outproj(1, cfp)
                attention(1, 3)
                allgather(3)
                outproj(2, cfp)
                outproj(3, cfp)

    nc.compile()
    return nc


def _build_nc():
    if "nc" not in _nc_cache:
        _nc_cache["nc"] = _build()
    return _nc_cache["nc"]


def kernel(x, W_q, W_k, W_v, W_o):
    x = np.ascontiguousarray(np.asarray(x, dtype=np.float32)).reshape(BS, D)
    # keep-mask for the diagonal 128x128 block of S^T[k, q]: keep k <= q
    tri = np.triu(np.ones((128, 128), dtype=np.float32))
    in_maps = []
    for c in range(N_CORES):
        sl = slice(c * DHC, (c + 1) * DHC)
        in_maps.append(
            {
                "x": x,
                "wq": np.ascontiguousarray(np.asarray(W_q, np.float32)[:, sl]),
                "wk": np.ascontiguousarray(np.asarray(W_k, np.float32)[:, sl]),
                "wv": np.ascontiguousarray(np.asarray(W_v, np.float32)[:, sl]),
                "wo": np.ascontiguousarray(np.asarray(W_o, np.float32)[:, sl]),
                "tri": tri,
            }
        )
    nc = _build_nc()
    res = run_bass_kernel_spmd(nc, in_maps, core_ids=list(range(N_CORES)))
    out = np.concatenate([res.results[c]["out"] for c in range(N_CORES)], axis=1)
    return out.reshape(B, S, D)
